# revision 2
# baseline (speedup 1.0000x reference)
"""Bahdanau additive attention kernel for Trainium2 (8 NeuronCores, SPMD).

Problem (hardcoded): B=32, Tq=4, S=2048, H=1024, 2H=2048, fp32 inputs.
  q  = query[:, -1, :]                      [B, H]
  k  = transpose(keys, (1, 0, 2))           [B, S, 2H]
  wq = q @ Wa_w.T + Wa_b                    [B, H]
  uk = k @ Ua_w.T + Ua_b                    [B, S, H]
  sc = tanh(wq[:, None, :] + uk) @ Va_w.T   [B, S]   (+ Va_b, which softmax cancels)
  w  = softmax(sc, axis=-1)                 [B, S]
  ctx = w @ k                               [B, 2H]
  returns (ctx [B,1,2H], w [B,1,S])

Sharding: data-parallel over batch. 8 cores x 4 batches each; weights
replicated; no cross-core communication.

Layout strategy: the host pre-arranges the small weight tensors into the
layouts the PE needs (Ua/Wa transposed so the contraction dim lands on
partitions, q/Va as column panels) and pre-casts them to bf16 -- pure
permutation/rounding marshalling, all model arithmetic stays on device.
This removes the DRAM->DRAM weight casts and xbar transposes that
previously serialized ~113us of startup.

Per-core dataflow (all matmuls bf16 with fp32 PSUM accumulation):
  - keys strips [128, 2H] are cast-loaded f32->bf16 on gpsimd (kept in
    SBUF for the context matmul) and staged to a DRAM scratch on the same
    gpsimd queue (keeps the scalar queue free for activations), then read
    back transposed per chunk via the DMA xbar ([d=128, s=512] tiles),
    prefetched two chunks ahead.
  - ukT tiles [h=128, s=512] accumulate in PSUM; ScalarE applies
    tanh(. + bias[h]) where bias = wq[b] + Wa_b + Ua_b folded per-partition.
  - scores via PE with Va columns as the 1-wide stationary operand.
  - softmax without max-subtraction (scores are O(1)); exp on ScalarE with
    free-dim accumulate for the denominator.
  - context via PE with normalized-late weights columns (tiny PE
    transposes of the score row) against the cached bf16 keys strips.
  - outputs DMA'd on the sync queue.
"""

import numpy as np

B, TQ, S, H = 32, 4, 2048, 1024
D2 = 2 * H
NCORES = 8
BPC = B // NCORES  # batches per core

_CACHE = {}


def _build(s=S, h=H, bpc=BPC, schunk=512):
    """Build the per-core Bass module. Parameterized so a scaled-down config
    can run in CoreSim; the shipped kernel uses the defaults."""
    from contextlib import ExitStack

    import concourse.bacc as bacc
    import concourse.bass as bass
    import concourse.mybir as mybir
    import concourse.tile as tile
    from concourse.masks import make_identity

    fp32 = mybir.dt.float32
    bf16 = mybir.dt.bfloat16
    AF = mybir.ActivationFunctionType
    d2 = 2 * h
    SD = d2 // 128        # contraction strips for uk (d on partitions)
    SM = h // 128         # h tiles (uk output partitions / Va strips)
    SJ = h // 128         # contraction strips for wq
    NCH = s // schunk     # score chunks per batch
    SPC = schunk // 128   # keys strips per chunk
    NDC = d2 // 512       # context output chunks
    NST = s // 128        # keys strips per batch
    NPOS = bpc * NCH      # total chunk positions

    nc = bacc.Bacc(
        "TRN2", target_bir_lowering=False, enable_partition_id=False
    )

    qt_in = nc.dram_tensor("qt", [128, SJ * bpc], bf16, kind="ExternalInput").ap()
    keys_in = nc.dram_tensor("keys", [s, bpc, d2], fp32, kind="ExternalInput").ap()
    wat_in = nc.dram_tensor("wat", [h, h], bf16, kind="ExternalInput").ap()
    uat_in = nc.dram_tensor("uat", [d2, h], bf16, kind="ExternalInput").ap()
    wab_in = nc.dram_tensor("wab", [1, h], bf16, kind="ExternalInput").ap()
    uab_in = nc.dram_tensor("uab", [1, h], bf16, kind="ExternalInput").ap()
    vac_in = nc.dram_tensor("vac", [128, SM], bf16, kind="ExternalInput").ap()
    ctx_out = nc.dram_tensor("ctx", [bpc, d2], fp32, kind="ExternalOutput").ap()
    w_out = nc.dram_tensor("wts", [bpc, s], fp32, kind="ExternalOutput").ap()

    with tile.TileContext(nc) as tc:
        with ExitStack() as ctx:
            consts = ctx.enter_context(tc.tile_pool(name="consts", bufs=1))
            dram_kn = ctx.enter_context(
                tc.tile_pool(name="dram_kn", bufs=3, space="DRAM")
            )
            kcache = ctx.enter_context(tc.tile_pool(name="kcache", bufs=3 * SPC))
            ktp = ctx.enter_context(tc.tile_pool(name="ktp", bufs=3 * SD))
            tp = ctx.enter_context(tc.tile_pool(name="tp", bufs=SM + 1))
            rows = ctx.enter_context(tc.tile_pool(name="rows", bufs=2))
            acc1 = ctx.enter_context(tc.tile_pool(name="acc1", bufs=2))
            ps_setup = ctx.enter_context(
                tc.tile_pool(name="ps_setup", bufs=1, space="PSUM")
            )
            ps_uk = ctx.enter_context(tc.tile_pool(name="ps_uk", bufs=3, space="PSUM"))
            ps_sc = ctx.enter_context(tc.tile_pool(name="ps_sc", bufs=2, space="PSUM"))
            ps_cx = ctx.enter_context(tc.tile_pool(name="ps_cx", bufs=2, space="PSUM"))

            # ---------------- keys pipeline helpers ----------------
            knats = {}
            pending_strips = {}
            pending_kts = {}

            def issue_chunk(b, c):
                # cast-load the chunk's strips (kept for the context matmul),
                # stage them to the DRAM scratch on the same gpsimd queue so
                # the store naturally follows its producer load in FIFO
                # order, then read back transposed on the sync queue.
                if b not in knats:
                    knats[b] = dram_kn.tile(
                        [s, d2], bf16, tag="knat", name=f"knat_b{b}"
                    )
                knat = knats[b]
                strips = []
                for i in range(SPC):
                    si = c * SPC + i
                    ks = kcache.tile([128, d2], bf16, tag="ks", name=f"ks_{b}_{si}")
                    nc.gpsimd.dma_start(
                        out=ks, in_=keys_in[si * 128 : (si + 1) * 128, b, :]
                    )
                    strips.append(ks)
                for i, ks in enumerate(strips):
                    si = c * SPC + i
                    nc.gpsimd.dma_start(
                        out=knat[si * 128 : (si + 1) * 128, :], in_=ks
                    )
                pending_strips[(b, c)] = strips
                kts = []
                for d in range(SD):
                    kt = ktp.tile(
                        [128, schunk], bf16, tag="kt", name=f"kt_{b}_{c}_{d}"
                    )
                    nc.sync.dma_start(
                        out=kt,
                        in_=knat[
                            c * schunk : (c + 1) * schunk, d * 128 : (d + 1) * 128
                        ],
                        transpose=True,
                    )
                    kts.append(kt)
                pending_kts[(b, c)] = kts

            # ---------------- one-time setup ----------------
            ident_f32 = consts.tile([128, 128], fp32)
            make_identity(nc, ident_f32)

            # keys chunk 0+1 pipelines start immediately (gpsimd + sync)
            issue_chunk(0, 0)
            if NCH > 1:
                issue_chunk(0, 1)

            # weight panels arrive pre-transposed/pre-cast; plain HWDGE
            # loads on the scalar queue (no SWDGE cast needed)
            qT = consts.tile([128, SJ, bpc], bf16)
            nc.scalar.dma_start(out=qT, in_=qt_in)
            va_cols = consts.tile([128, SM], bf16)
            nc.scalar.dma_start(out=va_cols, in_=vac_in)
            uab_row = consts.tile([1, h], bf16)
            nc.scalar.dma_start(out=uab_row, in_=uab_in)
            wab_row = consts.tile([1, h], bf16)
            nc.scalar.dma_start(out=wab_row, in_=wab_in)
            waT = consts.tile([128, SJ, h], bf16)
            for j in range(SJ):
                nc.scalar.dma_start(
                    out=waT[:, j, :], in_=wat_in[j * 128 : (j + 1) * 128, :]
                )
            uaT = consts.tile([128, SD, h], bf16)
            for d in range(SD):
                nc.scalar.dma_start(
                    out=uaT[:, d, :], in_=uat_in[d * 128 : (d + 1) * 128, :]
                )

            # combined additive bias row (Wa_b + Ua_b), bf16 for the K=1 matmul
            comb_bf = consts.tile([1, h], bf16)
            nc.vector.tensor_tensor(
                out=comb_bf, in0=uab_row, in1=wab_row, op=mybir.AluOpType.add
            )
            ones_bf = consts.tile([1, bpc], bf16)
            nc.vector.memset(ones_bf, 1.0)

            # bias_cols[:, m, b] = (Wa q_b)[128m:128m+128] + Wa_b + Ua_b  (fp32)
            bias_cols = consts.tile([128, SM, bpc], fp32)
            for m in range(SM):
                pw = ps_setup.tile([128, bpc], fp32, tag="setup")
                for j in range(SJ):
                    nc.tensor.matmul(
                        out=pw,
                        lhsT=waT[:, j, m * 128 : (m + 1) * 128],
                        rhs=qT[:, j, :],
                        start=(j == 0),
                        stop=False,
                    )
                nc.tensor.matmul(
                    out=pw,
                    lhsT=comb_bf[:1, m * 128 : (m + 1) * 128],
                    rhs=ones_bf,
                    start=False,
                    stop=True,
                )
                nc.vector.tensor_copy(out=bias_cols[:, m, :], in_=pw)

            # ---------------- main loop over batches ----------------
            for b in range(bpc):
                exp_row = rows.tile([1, s], fp32, tag="exp_row")
                tparts = rows.tile([1, NCH], fp32, tag="tparts")
                ecols = rows.tile([128, NST], bf16, tag="ecols")
                ctx_acc = acc1.tile([1, d2], fp32, tag="ctx_acc")
                for c in range(NCH):
                    pos = b * NCH + c
                    # prefetch the chunk two positions ahead
                    if pos + 2 < NPOS:
                        nxt = pos + 2
                        issue_chunk(nxt // NCH, nxt % NCH)
                    kts = pending_kts.pop((b, c))
                    strips = pending_strips.pop((b, c))
                    # ukT tiles + tanh; score matmuls are deferred until all
                    # tanh tiles exist so the in-order PE queue never waits
                    # on the Scalar engine mid-chunk
                    psc = ps_sc.tile([1, schunk], fp32, tag="psc")
                    ts_list = []
                    for m in range(SM):
                        puk = ps_uk.tile([128, schunk], fp32, tag="puk")
                        for d in range(SD):
                            nc.tensor.matmul(
                                out=puk,
                                lhsT=uaT[:, d, m * 128 : (m + 1) * 128],
                                rhs=kts[d],
                                start=(d == 0),
                                stop=(d == SD - 1),
                            )
                        t_sb = tp.tile([128, schunk], bf16, tag="t")
                        nc.scalar.activation(
                            out=t_sb,
                            in_=puk,
                            func=AF.Tanh,
                            bias=bias_cols[:, m, b : b + 1],
                            scale=1.0,
                        )
                        ts_list.append(t_sb)
                    for m in range(SM):
                        nc.tensor.matmul(
                            out=psc,
                            lhsT=va_cols[:, m : m + 1],
                            rhs=ts_list[m],
                            start=(m == 0),
                            stop=(m == SM - 1),
                        )
                    # exp row chunk (no max subtraction; scores are O(1)) and
                    # the chunk's softmax partial sum
                    nc.scalar.activation(
                        out=exp_row[:, c * schunk : (c + 1) * schunk],
                        in_=psc,
                        func=AF.Exp,
                        accum_out=tparts[:, c : c + 1],
                    )
                    # transpose this chunk's scores into columns on PE (tiny)
                    # and exp them -> unnormalized weight columns for context
                    scsb = rows.tile([1, schunk], fp32, tag="scsb")
                    nc.vector.tensor_copy(out=scsb, in_=psc)
                    pscT = ps_setup.tile([128, SPC], fp32, tag="setup")
                    for g in range(SPC):
                        nc.tensor.transpose(
                            out=pscT[:, g : g + 1],
                            in_=scsb[:1, g * 128 : (g + 1) * 128],
                            identity=ident_f32[:1, :1],
                        )
                    nc.scalar.activation(
                        out=ecols[:, c * SPC : (c + 1) * SPC],
                        in_=pscT,
                        func=AF.Exp,
                    )
                    # context partial for this chunk's strips (normalized at
                    # the end of the batch): ctx += sum_si e[si] * k[si, :]
                    for jd in range(NDC):
                        pcx = ps_cx.tile([1, 512], fp32, tag="pcx")
                        for i in range(SPC):
                            nc.tensor.matmul(
                                out=pcx,
                                lhsT=ecols[:, c * SPC + i : c * SPC + i + 1],
                                rhs=strips[i][:, jd * 512 : (jd + 1) * 512],
                                start=(i == 0),
                                stop=(i == SPC - 1),
                            )
                        if c == 0:
                            nc.vector.tensor_copy(
                                out=ctx_acc[:, jd * 512 : (jd + 1) * 512], in_=pcx
                            )
                        else:
                            nc.vector.tensor_add(
                                out=ctx_acc[:, jd * 512 : (jd + 1) * 512],
                                in0=ctx_acc[:, jd * 512 : (jd + 1) * 512],
                                in1=pcx,
                            )
                # softmax denominator; normalize weights + context, write out
                tsum = rows.tile([1, 1], fp32, tag="tsum")
                nc.vector.reduce_sum(
                    out=tsum, in_=tparts, axis=mybir.AxisListType.X
                )
                invt = rows.tile([1, 1], fp32, tag="invt")
                nc.vector.reciprocal(out=invt, in_=tsum)
                nc.vector.tensor_scalar_mul(out=exp_row, in0=exp_row, scalar1=invt)
                nc.sync.dma_start(out=w_out[b : b + 1, :], in_=exp_row)
                nc.vector.tensor_scalar_mul(out=ctx_acc, in0=ctx_acc, scalar1=invt)
                nc.sync.dma_start(out=ctx_out[b : b + 1, :], in_=ctx_acc)

    nc.compile()
    return nc


def _get_nc():
    if "nc" not in _CACHE:
        _CACHE["nc"] = _build()
    return _CACHE["nc"]


def _make_in_maps(inputs):
    import ml_dtypes

    bf16 = ml_dtypes.bfloat16
    SJ = H // 128
    SM = H // 128

    q_last = np.asarray(inputs["query"], dtype=np.float32)[:, -1, :]  # [B, H]
    keys = np.asarray(inputs["keys"], dtype=np.float32)  # [S, B, 2H]
    # weight panels: pre-transpose/pre-arrange + bf16 cast (layout
    # marshalling only; same rounding the device DMA cast applied)
    uaT = np.ascontiguousarray(
        np.asarray(inputs["Ua_w"], dtype=np.float32).T
    ).astype(bf16)  # [2H, H]
    waT = np.ascontiguousarray(
        np.asarray(inputs["Wa_w"], dtype=np.float32).T
    ).astype(bf16)  # [H, H]
    vac = np.ascontiguousarray(
        np.asarray(inputs["Va_w"], dtype=np.float32).reshape(SM, 128).T
    ).astype(bf16)  # [128, SM]
    uab = np.asarray(inputs["Ua_b"], dtype=np.float32).reshape(1, H).astype(bf16)
    wab = np.asarray(inputs["Wa_b"], dtype=np.float32).reshape(1, H).astype(bf16)

    in_maps = []
    for cidx in range(NCORES):
        b0 = cidx * BPC
        qt = np.ascontiguousarray(
            q_last[b0 : b0 + BPC]
            .T.reshape(SJ, 128, BPC)
            .transpose(1, 0, 2)
            .reshape(128, SJ * BPC)
        ).astype(bf16)
        in_maps.append(
            {
                "qt": qt,
                "keys": np.ascontiguousarray(keys[:, b0 : b0 + BPC, :]),
                "wat": waT,
                "uat": uaT,
                "wab": wab,
                "uab": uab,
                "vac": vac,
            }
        )
    return in_maps


def run(inputs, trace=False, **kwargs):
    """Run on all 8 cores; returns ((context, weights), BassKernelResults)."""
    from concourse.bass_utils import run_bass_kernel_spmd

    nc = _get_nc()
    in_maps = _make_in_maps(inputs)
    res = run_bass_kernel_spmd(
        nc, in_maps, core_ids=list(range(NCORES)), trace=trace, **kwargs
    )
    context = np.empty((B, 1, D2), dtype=np.float32)
    weights = np.empty((B, 1, S), dtype=np.float32)
    for c in range(NCORES):
        b0 = c * BPC
        context[b0 : b0 + BPC, 0, :] = res.results[c]["ctx"]
        weights[b0 : b0 + BPC, 0, :] = res.results[c]["wts"]
    return (context, weights), res


def kernel(**inputs):
    out, _ = run(inputs)
    return out


# revision 13
# speedup vs baseline: 1.0966x; 1.0966x over previous
"""Bahdanau additive attention kernel for Trainium2 (8 NeuronCores, SPMD).

Problem (hardcoded): B=32, Tq=4, S=2048, H=1024, 2H=2048, fp32 inputs.
  q  = query[:, -1, :]                      [B, H]
  k  = transpose(keys, (1, 0, 2))           [B, S, 2H]
  wq = q @ Wa_w.T + Wa_b                    [B, H]
  uk = k @ Ua_w.T + Ua_b                    [B, S, H]
  sc = tanh(wq[:, None, :] + uk) @ Va_w.T   [B, S]   (+ Va_b, which softmax cancels)
  w  = softmax(sc, axis=-1)                 [B, S]
  ctx = w @ k                               [B, 2H]
  returns (ctx [B,1,2H], w [B,1,S])

Sharding: data-parallel over batch. 8 cores x 4 batches each; weights
replicated; no cross-core communication.

Layout strategy: the host pre-arranges the small weight tensors into the
layouts the PE needs (Ua/Wa transposed so the contraction dim lands on
partitions, q/Va as column panels) and pre-casts them to bf16 -- pure
permutation/rounding marshalling, all model arithmetic stays on device.
This removes the DRAM->DRAM weight casts and xbar transposes that
previously serialized ~113us of startup.

Per-core dataflow (all matmuls bf16 with fp32 PSUM accumulation):
  - keys strips [128, 2H] are cast-loaded f32->bf16 on gpsimd (kept in
    SBUF for the context matmul) and staged to a DRAM scratch on the same
    gpsimd queue (keeps the scalar queue free for activations), then read
    back transposed per chunk via the DMA xbar ([d=128, s=512] tiles),
    prefetched two chunks ahead.
  - ukT tiles [h=128, s=512] accumulate in PSUM; ScalarE applies
    tanh(. + bias[h]) where bias = wq[b] + Wa_b + Ua_b folded per-partition.
  - scores via PE with Va columns as the 1-wide stationary operand.
  - softmax without max-subtraction (scores are O(1)); exp on ScalarE with
    free-dim accumulate for the denominator.
  - context via PE with normalized-late weights columns (tiny PE
    transposes of the score row) against the cached bf16 keys strips.
  - outputs DMA'd on the sync queue.
"""

import numpy as np

B, TQ, S, H = 32, 4, 2048, 1024
D2 = 2 * H
NCORES = 8
BPC = B // NCORES  # batches per core

_CACHE = {}


def _build(s=S, h=H, bpc=BPC, schunk=512):
    """Build the per-core Bass module. Parameterized so a scaled-down config
    can run in CoreSim; the shipped kernel uses the defaults."""
    from contextlib import ExitStack

    import concourse.bacc as bacc
    import concourse.bass as bass
    import concourse.mybir as mybir
    import concourse.tile as tile
    from concourse.masks import make_identity

    fp32 = mybir.dt.float32
    bf16 = mybir.dt.bfloat16
    AF = mybir.ActivationFunctionType
    d2 = 2 * h
    SD = d2 // 128        # contraction strips for uk (d on partitions)
    SM = h // 128         # h tiles (uk output partitions / Va strips)
    SJ = h // 128         # contraction strips for wq
    NCH = s // schunk     # score chunks per batch
    SPC = schunk // 128   # keys strips per chunk
    NDC = d2 // 512       # context output chunks
    NST = s // 128        # keys strips per batch
    NPOS = bpc * NCH      # total chunk positions

    nc = bacc.Bacc(
        "TRN2", target_bir_lowering=False, enable_partition_id=False
    )

    qt_in = nc.dram_tensor("qt", [128, SJ * bpc], bf16, kind="ExternalInput").ap()
    keys_in = nc.dram_tensor("keys", [s, bpc, d2], fp32, kind="ExternalInput").ap()
    wat_in = nc.dram_tensor("wat", [h, h], bf16, kind="ExternalInput").ap()
    uat_in = nc.dram_tensor("uat", [d2, h], bf16, kind="ExternalInput").ap()
    wab_in = nc.dram_tensor("wab", [1, h], bf16, kind="ExternalInput").ap()
    uab_in = nc.dram_tensor("uab", [1, h], bf16, kind="ExternalInput").ap()
    vac_in = nc.dram_tensor("vac", [128, SM], bf16, kind="ExternalInput").ap()
    ctx_out = nc.dram_tensor("ctx", [bpc, d2], fp32, kind="ExternalOutput").ap()
    w_out = nc.dram_tensor("wts", [bpc, s], fp32, kind="ExternalOutput").ap()

    with tile.TileContext(nc) as tc:
        with ExitStack() as ctx:
            consts = ctx.enter_context(tc.tile_pool(name="consts", bufs=1))
            dram_kn = ctx.enter_context(
                tc.tile_pool(name="dram_kn", bufs=3, space="DRAM")
            )
            kcache = ctx.enter_context(tc.tile_pool(name="kcache", bufs=4 * SPC))
            ktp = ctx.enter_context(tc.tile_pool(name="ktp", bufs=2 * SD))
            tp = ctx.enter_context(tc.tile_pool(name="tp", bufs=SM))
            rows = ctx.enter_context(tc.tile_pool(name="rows", bufs=2))
            acc1 = ctx.enter_context(tc.tile_pool(name="acc1", bufs=1))
            ps_setup = ctx.enter_context(
                tc.tile_pool(name="ps_setup", bufs=1, space="PSUM")
            )
            ps_uk = ctx.enter_context(tc.tile_pool(name="ps_uk", bufs=3, space="PSUM"))
            ps_sc = ctx.enter_context(tc.tile_pool(name="ps_sc", bufs=2, space="PSUM"))
            ps_cx = ctx.enter_context(tc.tile_pool(name="ps_cx", bufs=2, space="PSUM"))

            # ---------------- keys pipeline helpers ----------------
            LG = 2 if NCH % 2 == 0 else 1  # chunks per kT group
            knats = {}
            pending_strips = {}
            pending_kts = {}

            def issue_strips(b, c):
                # cast-load the chunk's strips (kept for the context matmul),
                # then stage them to the DRAM scratch on the same gpsimd
                # queue so each store naturally follows its producer load in
                # FIFO order (keeps the scalar queue free for activations)
                if b not in knats:
                    knats[b] = dram_kn.tile(
                        [s, d2], bf16, tag="knat", name=f"knat_b{b}"
                    )
                knat = knats[b]
                strips = []
                for i in range(SPC):
                    si = c * SPC + i
                    ks = kcache.tile([128, d2], bf16, tag="ks", name=f"ks_{b}_{si}")
                    nc.gpsimd.dma_start(
                        out=ks, in_=keys_in[si * 128 : (si + 1) * 128, b, :]
                    )
                    strips.append(ks)
                for i, ks in enumerate(strips):
                    si = c * SPC + i
                    nc.gpsimd.dma_start(
                        out=knat[si * 128 : (si + 1) * 128, :], in_=ks
                    )
                pending_strips[(b, c)] = strips

            def issue_kts(b, g, waves=1):
                # transposed [d=128, s=LG*schunk] tiles via the DMA xbar on
                # the sync queue; one xbar per d covering LG chunks (the
                # ~1.2us fixed cost per xbar makes fewer/bigger transposes
                # cheaper). waves=LG splits each tile fill into per-chunk
                # xbars so the first chunk's matmuls start sooner (startup).
                knat = knats[b]
                kts = []
                for d in range(SD):
                    kt = ktp.tile(
                        [128, LG * schunk], bf16, tag="kt", name=f"kt_{b}_{g}_{d}"
                    )
                    kts.append(kt)
                r0 = g * LG * schunk
                for w in range(waves):
                    rows_w = LG * schunk // waves
                    for d in range(SD):
                        nc.sync.dma_start(
                            out=kts[d][:, w * rows_w : (w + 1) * rows_w],
                            in_=knat[
                                r0 + w * rows_w : r0 + (w + 1) * rows_w,
                                d * 128 : (d + 1) * 128,
                            ],
                            transpose=True,
                        )
                pending_kts[(b, g)] = kts

            # ---------------- one-time setup ----------------
            ident_f32 = consts.tile([128, 128], fp32)
            make_identity(nc, ident_f32)

            # combined additive bias row (Wa_b + Ua_b) via accumulate-DMA;
            # issued first on gpsimd (tiny) since it gates chunk 0's tanh
            comb_bf = consts.tile([1, h], bf16)
            nc.gpsimd.dma_start(out=comb_bf, in_=uab_in)
            nc.gpsimd.dma_start(
                out=comb_bf, in_=wab_in, accum_op=mybir.AluOpType.add
            )

            # keys pipeline for the first group starts immediately; the
            # first group's xbars go per-chunk so chunk 0 can start as soon
            # as its own strips are staged
            issue_strips(0, 0)
            if NCH > 1:
                issue_strips(0, 1)
            issue_kts(0, 0, waves=LG)
            if NCH > 2:
                issue_strips(0, 2)

            # weight panels arrive pre-transposed/pre-cast; plain HWDGE
            # loads on the scalar queue (no SWDGE cast needed)
            qT = consts.tile([128, SJ, bpc], bf16)
            nc.scalar.dma_start(out=qT, in_=qt_in)
            va_cols = consts.tile([128, SM], bf16)
            nc.scalar.dma_start(out=va_cols, in_=vac_in)
            waT = consts.tile([128, SJ, h], bf16)
            for j in range(SJ):
                nc.scalar.dma_start(
                    out=waT[:, j, :], in_=wat_in[j * 128 : (j + 1) * 128, :]
                )
            uaT = consts.tile([128, SD, h], bf16)
            for d in range(SD):
                nc.scalar.dma_start(
                    out=uaT[:, d, :], in_=uat_in[d * 128 : (d + 1) * 128, :]
                )

            ones_bf = consts.tile([1, bpc], bf16)
            nc.vector.memset(ones_bf, 1.0)

            # bias_cols[:, m, b] = (Wa q_b)[128m:128m+128] + Wa_b + Ua_b  (fp32)
            bias_cols = consts.tile([128, SM, bpc], fp32)
            for m in range(SM):
                pw = ps_setup.tile([128, bpc], fp32, tag="setup")
                for j in range(SJ):
                    nc.tensor.matmul(
                        out=pw,
                        lhsT=waT[:, j, m * 128 : (m + 1) * 128],
                        rhs=qT[:, j, :],
                        start=(j == 0),
                        stop=False,
                    )
                nc.tensor.matmul(
                    out=pw,
                    lhsT=comb_bf[:1, m * 128 : (m + 1) * 128],
                    rhs=ones_bf,
                    start=False,
                    stop=True,
                )
                nc.vector.tensor_copy(out=bias_cols[:, m, :], in_=pw)

            # ---------------- main loop over batches ----------------
            for b in range(bpc):
                exp_row = rows.tile([1, s], fp32, tag="exp_row", bufs=1)
                tparts = rows.tile([1, NCH], fp32, tag="tparts")
                ecols = rows.tile([128, NST], bf16, tag="ecols")
                ctx_acc = acc1.tile([1, d2], fp32, tag="ctx_acc")
                for c in range(NCH):
                    pos = b * NCH + c
                    # prefetch strips three chunks ahead; at each group
                    # boundary issue the next group's xbars (they sem-wait
                    # on the staging stores and land a chunk before use)
                    if pos + 3 < NPOS:
                        nxt = pos + 3
                        issue_strips(nxt // NCH, nxt % NCH)
                    if c % LG == 0 and pos + LG < NPOS:
                        ngp = pos + LG
                        issue_kts(ngp // NCH, (ngp % NCH) // LG)
                    kts_group = pending_kts.pop((b, c // LG)) if c % LG == 0 else kts_group
                    kts = [kt[:, (c % LG) * schunk : (c % LG + 1) * schunk] for kt in kts_group]
                    strips = pending_strips.pop((b, c))
                    # ukT tiles + tanh; score matmuls are deferred until all
                    # tanh tiles exist so the in-order PE queue never waits
                    # on the Scalar engine mid-chunk
                    psc = ps_sc.tile([1, schunk], fp32, tag="psc")
                    ts_list = []
                    for m in range(SM):
                        puk = ps_uk.tile([128, schunk], fp32, tag="puk")
                        for d in range(SD):
                            nc.tensor.matmul(
                                out=puk,
                                lhsT=uaT[:, d, m * 128 : (m + 1) * 128],
                                rhs=kts[d],
                                start=(d == 0),
                                stop=(d == SD - 1),
                            )
                        t_sb = tp.tile([128, schunk], bf16, tag="t")
                        nc.scalar.activation(
                            out=t_sb,
                            in_=puk,
                            func=AF.Tanh,
                            bias=bias_cols[:, m, b : b + 1],
                            scale=1.0,
                        )
                        ts_list.append(t_sb)
                    for m in range(SM):
                        nc.tensor.matmul(
                            out=psc,
                            lhsT=va_cols[:, m : m + 1],
                            rhs=ts_list[m],
                            start=(m == 0),
                            stop=(m == SM - 1),
                        )
                    # exp row chunk (no max subtraction; scores are O(1)) and
                    # the chunk's softmax partial sum
                    nc.scalar.activation(
                        out=exp_row[:, c * schunk : (c + 1) * schunk],
                        in_=psc,
                        func=AF.Exp,
                        accum_out=tparts[:, c : c + 1],
                    )
                    # transpose this chunk's scores into columns on PE (tiny)
                    # and exp them -> unnormalized weight columns for context
                    scsb = rows.tile([1, schunk], fp32, tag="scsb", bufs=1)
                    nc.vector.tensor_copy(out=scsb, in_=psc)
                    pscT = ps_setup.tile([128, SPC], fp32, tag="setup")
                    for g in range(SPC):
                        nc.tensor.transpose(
                            out=pscT[:, g : g + 1],
                            in_=scsb[:1, g * 128 : (g + 1) * 128],
                            identity=ident_f32[:1, :1],
                        )
                    nc.scalar.activation(
                        out=ecols[:, c * SPC : (c + 1) * SPC],
                        in_=pscT,
                        func=AF.Exp,
                    )
                    # context partial for this chunk's strips (normalized at
                    # the end of the batch): ctx += sum_si e[si] * k[si, :]
                    for jd in range(NDC):
                        pcx = ps_cx.tile([1, 512], fp32, tag="pcx")
                        for i in range(SPC):
                            nc.tensor.matmul(
                                out=pcx,
                                lhsT=ecols[:, c * SPC + i : c * SPC + i + 1],
                                rhs=strips[i][:, jd * 512 : (jd + 1) * 512],
                                start=(i == 0),
                                stop=(i == SPC - 1),
                            )
                        if c == 0:
                            nc.vector.tensor_copy(
                                out=ctx_acc[:, jd * 512 : (jd + 1) * 512], in_=pcx
                            )
                        else:
                            nc.vector.tensor_add(
                                out=ctx_acc[:, jd * 512 : (jd + 1) * 512],
                                in0=ctx_acc[:, jd * 512 : (jd + 1) * 512],
                                in1=pcx,
                            )
                # softmax denominator; normalize weights + context, write out
                tsum = rows.tile([1, 1], fp32, tag="tsum")
                nc.vector.reduce_sum(
                    out=tsum, in_=tparts, axis=mybir.AxisListType.X
                )
                invt = rows.tile([1, 1], fp32, tag="invt")
                nc.vector.reciprocal(out=invt, in_=tsum)
                nc.vector.tensor_scalar_mul(out=exp_row, in0=exp_row, scalar1=invt)
                nc.sync.dma_start(out=w_out[b : b + 1, :], in_=exp_row)
                nc.vector.tensor_scalar_mul(out=ctx_acc, in0=ctx_acc, scalar1=invt)
                nc.sync.dma_start(out=ctx_out[b : b + 1, :], in_=ctx_acc)

    nc.compile()
    return nc


def _get_nc():
    if "nc" not in _CACHE:
        _CACHE["nc"] = _build()
    return _CACHE["nc"]


def _make_in_maps(inputs):
    import ml_dtypes

    bf16 = ml_dtypes.bfloat16
    SJ = H // 128
    SM = H // 128

    q_last = np.asarray(inputs["query"], dtype=np.float32)[:, -1, :]  # [B, H]
    keys = np.asarray(inputs["keys"], dtype=np.float32)  # [S, B, 2H]
    # weight panels: pre-transpose/pre-arrange + bf16 cast (layout
    # marshalling only; same rounding the device DMA cast applied)
    uaT = np.ascontiguousarray(
        np.asarray(inputs["Ua_w"], dtype=np.float32).T
    ).astype(bf16)  # [2H, H]
    waT = np.ascontiguousarray(
        np.asarray(inputs["Wa_w"], dtype=np.float32).T
    ).astype(bf16)  # [H, H]
    vac = np.ascontiguousarray(
        np.asarray(inputs["Va_w"], dtype=np.float32).reshape(SM, 128).T
    ).astype(bf16)  # [128, SM]
    uab = np.asarray(inputs["Ua_b"], dtype=np.float32).reshape(1, H).astype(bf16)
    wab = np.asarray(inputs["Wa_b"], dtype=np.float32).reshape(1, H).astype(bf16)

    in_maps = []
    for cidx in range(NCORES):
        b0 = cidx * BPC
        qt = np.ascontiguousarray(
            q_last[b0 : b0 + BPC]
            .T.reshape(SJ, 128, BPC)
            .transpose(1, 0, 2)
            .reshape(128, SJ * BPC)
        ).astype(bf16)
        in_maps.append(
            {
                "qt": qt,
                "keys": np.ascontiguousarray(keys[:, b0 : b0 + BPC, :]),
                "wat": waT,
                "uat": uaT,
                "wab": wab,
                "uab": uab,
                "vac": vac,
            }
        )
    return in_maps


def run(inputs, trace=False, **kwargs):
    """Run on all 8 cores; returns ((context, weights), BassKernelResults)."""
    from concourse.bass_utils import run_bass_kernel_spmd

    nc = _get_nc()
    in_maps = _make_in_maps(inputs)
    res = run_bass_kernel_spmd(
        nc, in_maps, core_ids=list(range(NCORES)), trace=trace, **kwargs
    )
    context = np.empty((B, 1, D2), dtype=np.float32)
    weights = np.empty((B, 1, S), dtype=np.float32)
    for c in range(NCORES):
        b0 = c * BPC
        context[b0 : b0 + BPC, 0, :] = res.results[c]["ctx"]
        weights[b0 : b0 + BPC, 0, :] = res.results[c]["wts"]
    return (context, weights), res


def kernel(**inputs):
    out, _ = run(inputs)
    return out


# revision 20
# speedup vs baseline: 1.0990x; 1.0022x over previous
"""Bahdanau additive attention kernel for Trainium2 (8 NeuronCores, SPMD).

Problem (hardcoded): B=32, Tq=4, S=2048, H=1024, 2H=2048, fp32 inputs.
  q  = query[:, -1, :]                      [B, H]
  k  = transpose(keys, (1, 0, 2))           [B, S, 2H]
  wq = q @ Wa_w.T + Wa_b                    [B, H]
  uk = k @ Ua_w.T + Ua_b                    [B, S, H]
  sc = tanh(wq[:, None, :] + uk) @ Va_w.T   [B, S]   (+ Va_b, which softmax cancels)
  w  = softmax(sc, axis=-1)                 [B, S]
  ctx = w @ k                               [B, 2H]
  returns (ctx [B,1,2H], w [B,1,S])

Sharding: data-parallel over batch. 8 cores x 4 batches each; weights
replicated; no cross-core communication.

Layout strategy: the host pre-arranges the small weight tensors into the
layouts the PE needs (Ua/Wa transposed so the contraction dim lands on
partitions, q/Va as column panels) and pre-casts them to bf16 -- pure
permutation/rounding marshalling, all model arithmetic stays on device.
This removes the DRAM->DRAM weight casts and xbar transposes that
previously serialized ~113us of startup.

Per-core dataflow (all matmuls bf16 with fp32 PSUM accumulation):
  - keys strips [128, 2H] are cast-loaded f32->bf16 on gpsimd (kept in
    SBUF for the context matmul) and staged to a DRAM scratch on the same
    gpsimd queue (keeps the scalar queue free for activations), then read
    back transposed per chunk via the DMA xbar ([d=128, s=512] tiles),
    prefetched two chunks ahead.
  - ukT tiles [h=128, s=512] accumulate in PSUM; ScalarE applies
    tanh(. + bias[h]) where bias = wq[b] + Wa_b + Ua_b folded per-partition.
  - scores via PE with Va columns as the 1-wide stationary operand.
  - softmax without max-subtraction (scores are O(1)); exp on ScalarE with
    free-dim accumulate for the denominator.
  - context via PE with normalized-late weights columns (tiny PE
    transposes of the score row) against the cached bf16 keys strips.
  - outputs DMA'd on the sync queue.
"""

import numpy as np

B, TQ, S, H = 32, 4, 2048, 1024
D2 = 2 * H
NCORES = 8
BPC = B // NCORES  # batches per core

_CACHE = {}


def _build(s=S, h=H, bpc=BPC, schunk=512):
    """Build the per-core Bass module. Parameterized so a scaled-down config
    can run in CoreSim; the shipped kernel uses the defaults."""
    from contextlib import ExitStack

    import concourse.bacc as bacc
    import concourse.bass as bass
    import concourse.mybir as mybir
    import concourse.tile as tile
    from concourse.masks import make_identity

    fp32 = mybir.dt.float32
    bf16 = mybir.dt.bfloat16
    AF = mybir.ActivationFunctionType
    d2 = 2 * h
    SD = d2 // 128        # contraction strips for uk (d on partitions)
    SM = h // 128         # h tiles (uk output partitions / Va strips)
    SJ = h // 128         # contraction strips for wq
    NCH = s // schunk     # score chunks per batch
    SPC = schunk // 128   # keys strips per chunk
    NDC = d2 // 512       # context output chunks
    NST = s // 128        # keys strips per batch
    NPOS = bpc * NCH      # total chunk positions

    nc = bacc.Bacc(
        "TRN2", target_bir_lowering=False, enable_partition_id=False
    )

    qt_in = nc.dram_tensor("qt", [128, SJ * bpc], bf16, kind="ExternalInput").ap()
    keys_in = nc.dram_tensor("keys", [s, bpc, d2], fp32, kind="ExternalInput").ap()
    wat_in = nc.dram_tensor("wat", [h, h], bf16, kind="ExternalInput").ap()
    uat_in = nc.dram_tensor("uat", [d2, h], bf16, kind="ExternalInput").ap()
    wab_in = nc.dram_tensor("wab", [1, h], bf16, kind="ExternalInput").ap()
    uab_in = nc.dram_tensor("uab", [1, h], bf16, kind="ExternalInput").ap()
    vac_in = nc.dram_tensor("vac", [128, SM], bf16, kind="ExternalInput").ap()
    ctx_out = nc.dram_tensor("ctx", [bpc, d2], fp32, kind="ExternalOutput").ap()
    w_out = nc.dram_tensor("wts", [bpc, s], fp32, kind="ExternalOutput").ap()

    with tile.TileContext(nc) as tc:
        with ExitStack() as ctx:
            consts = ctx.enter_context(tc.tile_pool(name="consts", bufs=1))
            dram_kn = ctx.enter_context(
                tc.tile_pool(name="dram_kn", bufs=3, space="DRAM")
            )
            kcache = ctx.enter_context(tc.tile_pool(name="kcache", bufs=4 * SPC))
            ktp = ctx.enter_context(tc.tile_pool(name="ktp", bufs=2 * SD))
            tp = ctx.enter_context(tc.tile_pool(name="tp", bufs=SM))
            rows = ctx.enter_context(tc.tile_pool(name="rows", bufs=2))
            acc1 = ctx.enter_context(tc.tile_pool(name="acc1", bufs=1))
            ps_setup = ctx.enter_context(
                tc.tile_pool(name="ps_setup", bufs=1, space="PSUM")
            )
            ps_uk = ctx.enter_context(tc.tile_pool(name="ps_uk", bufs=3, space="PSUM"))
            ps_sc = ctx.enter_context(tc.tile_pool(name="ps_sc", bufs=2, space="PSUM"))
            ps_cx = ctx.enter_context(tc.tile_pool(name="ps_cx", bufs=2, space="PSUM"))

            # ---------------- keys pipeline helpers ----------------
            LG = 2 if NCH % 2 == 0 else 1  # chunks per kT group
            knats = {}
            pending_strips = {}
            pending_kts = {}

            def issue_strips(b, c):
                # cast-load the chunk's strips (kept for the context matmul),
                # then stage them to the DRAM scratch on the same gpsimd
                # queue so each store naturally follows its producer load in
                # FIFO order (keeps the scalar queue free for activations)
                if b not in knats:
                    knats[b] = dram_kn.tile(
                        [s, d2], bf16, tag="knat", name=f"knat_b{b}"
                    )
                knat = knats[b]
                strips = []
                for i in range(SPC):
                    si = c * SPC + i
                    ks = kcache.tile([128, d2], bf16, tag="ks", name=f"ks_{b}_{si}")
                    nc.gpsimd.dma_start(
                        out=ks, in_=keys_in[si * 128 : (si + 1) * 128, b, :]
                    )
                    strips.append(ks)
                for i, ks in enumerate(strips):
                    si = c * SPC + i
                    nc.gpsimd.dma_start(
                        out=knat[si * 128 : (si + 1) * 128, :], in_=ks
                    )
                pending_strips[(b, c)] = strips

            def issue_kts(b, g, waves=1):
                # transposed [d=128, s=LG*schunk] tiles via the DMA xbar on
                # the sync queue; one xbar per d covering LG chunks (the
                # ~1.2us fixed cost per xbar makes fewer/bigger transposes
                # cheaper). waves=LG splits each tile fill into per-chunk
                # xbars so the first chunk's matmuls start sooner (startup).
                knat = knats[b]
                kts = []
                for d in range(SD):
                    kt = ktp.tile(
                        [128, LG * schunk], bf16, tag="kt", name=f"kt_{b}_{g}_{d}"
                    )
                    kts.append(kt)
                r0 = g * LG * schunk
                for w in range(waves):
                    rows_w = LG * schunk // waves
                    for d in range(SD):
                        nc.sync.dma_start(
                            out=kts[d][:, w * rows_w : (w + 1) * rows_w],
                            in_=knat[
                                r0 + w * rows_w : r0 + (w + 1) * rows_w,
                                d * 128 : (d + 1) * 128,
                            ],
                            transpose=True,
                        )
                pending_kts[(b, g)] = kts

            # ---------------- one-time setup ----------------
            ident_f32 = consts.tile([128, 128], fp32)
            make_identity(nc, ident_f32)

            # combined additive bias row (Wa_b + Ua_b) via accumulate-DMA;
            # issued first on gpsimd (tiny) since it gates chunk 0's tanh
            comb_bf = consts.tile([1, h], bf16)
            nc.gpsimd.dma_start(out=comb_bf, in_=uab_in)
            nc.gpsimd.dma_start(
                out=comb_bf, in_=wab_in, accum_op=mybir.AluOpType.add
            )

            # keys pipeline for the first group starts immediately; the
            # first group's xbars go per-chunk so chunk 0 can start as soon
            # as its own strips are staged
            issue_strips(0, 0)
            if NCH > 1:
                issue_strips(0, 1)
            issue_kts(0, 0, waves=LG)
            if NCH > 2:
                issue_strips(0, 2)

            # weight panels arrive pre-transposed/pre-cast; plain HWDGE
            # loads on the scalar queue (no SWDGE cast needed)
            qT = consts.tile([128, SJ, bpc], bf16)
            nc.scalar.dma_start(out=qT, in_=qt_in)
            va_cols = consts.tile([128, SM], bf16)
            nc.scalar.dma_start(out=va_cols, in_=vac_in)
            waT = consts.tile([128, SJ, h], bf16)
            for j in range(SJ):
                nc.scalar.dma_start(
                    out=waT[:, j, :], in_=wat_in[j * 128 : (j + 1) * 128, :]
                )
            uaT = consts.tile([128, SD, h], bf16)
            for d in range(SD):
                nc.scalar.dma_start(
                    out=uaT[:, d, :], in_=uat_in[d * 128 : (d + 1) * 128, :]
                )

            ones_bf = consts.tile([1, bpc], bf16)
            nc.vector.memset(ones_bf, 1.0)

            # bias_cols[:, m, b] = (Wa q_b)[128m:128m+128] + Wa_b + Ua_b  (fp32)
            bias_cols = consts.tile([128, SM, bpc], fp32)
            for m in range(SM):
                pw = ps_setup.tile([128, bpc], fp32, tag="setup")
                for j in range(SJ):
                    nc.tensor.matmul(
                        out=pw,
                        lhsT=waT[:, j, m * 128 : (m + 1) * 128],
                        rhs=qT[:, j, :],
                        start=(j == 0),
                        stop=False,
                    )
                nc.tensor.matmul(
                    out=pw,
                    lhsT=comb_bf[:1, m * 128 : (m + 1) * 128],
                    rhs=ones_bf,
                    start=False,
                    stop=True,
                )
                nc.vector.tensor_copy(out=bias_cols[:, m, :], in_=pw)

            # ---------------- main loop over batches ----------------
            for b in range(bpc):
                exp_row = rows.tile([1, s], fp32, tag="exp_row", bufs=1)
                tparts = rows.tile([1, NCH], fp32, tag="tparts")
                ecols = rows.tile([128, NST], bf16, tag="ecols")
                ctx_acc = acc1.tile([1, d2], fp32, tag="ctx_acc")
                for c in range(NCH):
                    pos = b * NCH + c
                    # prefetch strips three chunks ahead; at each group
                    # boundary issue the next group's xbars (they sem-wait
                    # on the staging stores and land a chunk before use)
                    if pos + 3 < NPOS:
                        nxt = pos + 3
                        issue_strips(nxt // NCH, nxt % NCH)
                    if c % LG == 0 and pos + LG < NPOS:
                        ngp = pos + LG
                        issue_kts(ngp // NCH, (ngp % NCH) // LG)
                    kts_group = pending_kts.pop((b, c // LG)) if c % LG == 0 else kts_group
                    kts = [kt[:, (c % LG) * schunk : (c % LG + 1) * schunk] for kt in kts_group]
                    strips = pending_strips.pop((b, c))
                    # ukT tiles + tanh; score matmuls are deferred until all
                    # tanh tiles exist so the in-order PE queue never waits
                    # on the Scalar engine mid-chunk
                    psc = ps_sc.tile([1, schunk], fp32, tag="psc")
                    ts_list = []
                    for m in range(SM):
                        puk = ps_uk.tile([128, schunk], fp32, tag="puk")
                        for d in range(SD):
                            nc.tensor.matmul(
                                out=puk,
                                lhsT=uaT[:, d, m * 128 : (m + 1) * 128],
                                rhs=kts[d],
                                start=(d == 0),
                                stop=(d == SD - 1),
                            )
                        t_sb = tp.tile([128, schunk], bf16, tag="t")
                        nc.scalar.activation(
                            out=t_sb,
                            in_=puk,
                            func=AF.Tanh,
                            bias=bias_cols[:, m, b : b + 1],
                            scale=1.0,
                        )
                        ts_list.append(t_sb)
                    for m in range(SM):
                        nc.tensor.matmul(
                            out=psc,
                            lhsT=va_cols[:, m : m + 1],
                            rhs=ts_list[m],
                            start=(m == 0),
                            stop=(m == SM - 1),
                        )
                    # exp row chunk (no max subtraction; scores are O(1)) and
                    # the chunk's softmax partial sum
                    nc.scalar.activation(
                        out=exp_row[:, c * schunk : (c + 1) * schunk],
                        in_=psc,
                        func=AF.Exp,
                        accum_out=tparts[:, c : c + 1],
                    )
                    # transpose this chunk's scores into columns on PE (tiny)
                    # and exp them -> unnormalized weight columns for context
                    scsb = rows.tile([1, schunk], fp32, tag="scsb", bufs=1)
                    nc.vector.tensor_copy(out=scsb, in_=psc)
                    pscT = ps_setup.tile([128, SPC], fp32, tag="setup")
                    for g in range(SPC):
                        nc.tensor.transpose(
                            out=pscT[:, g : g + 1],
                            in_=scsb[:1, g * 128 : (g + 1) * 128],
                            identity=ident_f32[:1, :1],
                        )
                    nc.scalar.activation(
                        out=ecols[:, c * SPC : (c + 1) * SPC],
                        in_=pscT,
                        func=AF.Exp,
                    )
                    # context partial for this chunk's strips (normalized at
                    # the end of the batch): ctx += sum_si e[si] * k[si, :]
                    for jd in range(NDC):
                        pcx = ps_cx.tile([1, 512], fp32, tag="pcx")
                        for i in range(SPC):
                            nc.tensor.matmul(
                                out=pcx,
                                lhsT=ecols[:, c * SPC + i : c * SPC + i + 1],
                                rhs=strips[i][:, jd * 512 : (jd + 1) * 512],
                                start=(i == 0),
                                stop=(i == SPC - 1),
                            )
                        if c == 0:
                            nc.vector.tensor_copy(
                                out=ctx_acc[:, jd * 512 : (jd + 1) * 512], in_=pcx
                            )
                        else:
                            nc.vector.tensor_add(
                                out=ctx_acc[:, jd * 512 : (jd + 1) * 512],
                                in0=ctx_acc[:, jd * 512 : (jd + 1) * 512],
                                in1=pcx,
                            )
                # softmax denominator; normalize weights + context, write out
                tsum = rows.tile([1, 1], fp32, tag="tsum")
                nc.vector.reduce_sum(
                    out=tsum, in_=tparts, axis=mybir.AxisListType.X
                )
                invt = rows.tile([1, 1], fp32, tag="invt")
                nc.vector.reciprocal(out=invt, in_=tsum)
                nc.vector.tensor_scalar_mul(out=exp_row, in0=exp_row, scalar1=invt)
                nc.sync.dma_start(out=w_out[b : b + 1, :], in_=exp_row)
                nc.vector.tensor_scalar_mul(out=ctx_acc, in0=ctx_acc, scalar1=invt)
                nc.sync.dma_start(out=ctx_out[b : b + 1, :], in_=ctx_acc)

    nc.compile()
    return nc


def _get_nc():
    if "nc" not in _CACHE:
        _CACHE["nc"] = _build()
    return _CACHE["nc"]


def _make_in_maps(inputs):
    import ml_dtypes

    bf16 = ml_dtypes.bfloat16
    SJ = H // 128
    SM = H // 128

    q_last = np.asarray(inputs["query"], dtype=np.float32)[:, -1, :]  # [B, H]
    keys = np.asarray(inputs["keys"], dtype=np.float32)  # [S, B, 2H]
    # weight panels: pre-transpose/pre-arrange + bf16 cast (layout
    # marshalling only; same rounding the device DMA cast applied)
    uaT = np.ascontiguousarray(
        np.asarray(inputs["Ua_w"], dtype=np.float32).T
    ).astype(bf16)  # [2H, H]
    waT = np.ascontiguousarray(
        np.asarray(inputs["Wa_w"], dtype=np.float32).T
    ).astype(bf16)  # [H, H]
    vac = np.ascontiguousarray(
        np.asarray(inputs["Va_w"], dtype=np.float32).reshape(SM, 128).T
    ).astype(bf16)  # [128, SM]
    uab = np.asarray(inputs["Ua_b"], dtype=np.float32).reshape(1, H).astype(bf16)
    wab = np.asarray(inputs["Wa_b"], dtype=np.float32).reshape(1, H).astype(bf16)

    in_maps = []
    for cidx in range(NCORES):
        b0 = cidx * BPC
        qt = np.ascontiguousarray(
            q_last[b0 : b0 + BPC]
            .T.reshape(SJ, 128, BPC)
            .transpose(1, 0, 2)
            .reshape(128, SJ * BPC)
        ).astype(bf16)
        in_maps.append(
            {
                "qt": qt,
                "keys": np.ascontiguousarray(keys[:, b0 : b0 + BPC, :]),
                "wat": waT,
                "uat": uaT,
                "wab": wab,
                "uab": uab,
                "vac": vac,
            }
        )
    return in_maps


def run(inputs, trace=False, **kwargs):
    """Run on all 8 cores; returns ((context, weights), BassKernelResults)."""
    from concourse.bass_utils import run_bass_kernel_spmd

    nc = _get_nc()
    in_maps = _make_in_maps(inputs)
    res = run_bass_kernel_spmd(
        nc, in_maps, core_ids=list(range(NCORES)), trace=trace, **kwargs
    )
    context = np.empty((B, 1, D2), dtype=np.float32)
    weights = np.empty((B, 1, S), dtype=np.float32)
    for c in range(NCORES):
        b0 = c * BPC
        context[b0 : b0 + BPC, 0, :] = res.results[c]["ctx"]
        weights[b0 : b0 + BPC, 0, :] = res.results[c]["wts"]
    return (context, weights), res


def kernel(**inputs):
    out, _ = run(inputs)
    return out
